# revision 8
# baseline (speedup 1.0000x reference)
"""AttnBlock (GroupNorm -> 1x1 q conv -> cross-attn over silu(nd)@W -> 1x1 proj -> residual)
for Trainium2, 8 NeuronCores, pure data parallel (2 batches per core).

Math (per batch b):
  hn   = GroupNorm(x)                            [C, HW]
  q    = q_w @ hn + q_b                          [C, HW]   (C on partitions)
  kv   = silu(nd) @ nd_w.T + nd_b                [L, C]
  lgT  = kv^T q * C^-1/2                         [L, HW]   (logits, transposed)
  attn = softmax over L
  out  = proj_w @ (kv^T attn) + proj_b ; y = x + out

Device-side algebra (all biases/affines folded into matmuls):
  - GroupNorm affine folded into q_w:  q2_w[c,o] = q_w.T[c,o]*a[c],
    qb2[o] = sum_c q_w.T[c,o]*bshift[c] + q_b[o]*sc  where
    a[c] = gamma[c]*rstd[g(c)]*sc, bshift[c] = (beta[c]-mean[g]*gamma[c]*rstd[g])*sc.
    So q comes straight from raw x (one matmul + bias) and carries the C^-0.5 scale.
  - logits computed transposed: lgT[l,n] = sum_c kv[c,l]*q[c,n]  (kv biased).
  - softmax denom: ones[128,128] matmul over exp tiles -> sums replicated on
    all 128 partitions; reciprocal_approx_fast -> r[n].
  - attnV and proj fused: pkv[l,o] = sum_c kv[c,l]*proj_w.T[c,o];
    o2[o,n] = sum_l pkv[l,o]*exp[l,n].  Then
    y = (o2*r + proj_b) + x   (nd_b bias term materializes exactly through the
    r normalization: (kv+nd_b) makes o2 pick up (proj_w@nd_b)[o]*sums[n]).
  - float32r (TF32-class single-pass PE mode) for all N>=128 matmuls.
"""

import numpy as np

B, C, HW = 16, 128, 4096
H = W = 64
L, ND = 512, 256
GROUPS = 32
EPS = 1e-6
NCORES = 8
NB = B // NCORES  # batches per core
SC = float(C) ** -0.5
NCHUNK = HW // 512  # 8 spatial chunks of 512
NL = L // 128       # 4 l-chunks of 128

_CACHE = {}


def _build():
    """Build the Bass module (one NeuronCore program, SPMD across 8 cores)."""
    from contextlib import ExitStack

    import concourse.bacc as bacc
    import concourse.bass as bass
    import concourse.mybir as mybir
    import concourse.tile as tile

    f32 = mybir.dt.float32
    f32r = mybir.dt.float32r
    Alu = mybir.AluOpType
    Act = mybir.ActivationFunctionType

    nc = bacc.Bacc(
        "TRN2",
        target_bir_lowering=False,
        debug=False,
        enable_asserts=False,
    )

    x_d = nc.dram_tensor("x", [NB, C, HW], f32, kind="ExternalInput").ap()
    nd_d = nc.dram_tensor("nd", [NB, L, ND], f32, kind="ExternalInput").ap()
    qwT_d = nc.dram_tensor("qwT", [C, C], f32, kind="ExternalInput").ap()
    pwT_d = nc.dram_tensor("pwT", [C, C], f32, kind="ExternalInput").ap()
    ndwT_d = nc.dram_tensor("ndwT", [ND, C], f32, kind="ExternalInput").ap()
    vec_d = nc.dram_tensor("vecs", [C, 8], f32, kind="ExternalInput").ap()
    ident_d = nc.dram_tensor("ident", [128, 128], f32, kind="ExternalInput").ap()
    ones_d = nc.dram_tensor("ones", [128, 128], f32, kind="ExternalInput").ap()
    ind4_d = nc.dram_tensor("ind4", [C, GROUPS], f32, kind="ExternalInput").ap()
    indT_d = nc.dram_tensor("indT", [GROUPS, C], f32, kind="ExternalInput").ap()
    y_d = nc.dram_tensor("y", [NB, C, HW], f32, kind="ExternalOutput").ap()

    import os
    use_f32r = os.environ.get("K_USE_F32R", "1") == "1"
    reps = int(os.environ.get("K_REPS", "1"))

    def r(ap):
        return ap.bitcast(f32r) if use_f32r else ap

    with tile.TileContext(nc) as tc:
        with ExitStack() as ctx:
            cpool = ctx.enter_context(tc.tile_pool(name="consts", bufs=1))
            xpool = ctx.enter_context(tc.tile_pool(name="xq", bufs=2))
            spool = ctx.enter_context(tc.tile_pool(name="small", bufs=2))
            apool = ctx.enter_context(tc.tile_pool(name="attn", bufs=2))
            ppool = ctx.enter_context(
                tc.tile_pool(name="psum", bufs=2, space="PSUM")
            )

            # ---- constants (loaded once) ----
            qwT = cpool.tile([C, C], f32)
            nc.sync.dma_start(qwT[:], qwT_d[:])
            pwT = cpool.tile([C, C], f32)
            nc.sync.dma_start(r(pwT[:]), r(pwT_d[:]))
            ndwT = cpool.tile([128, 2 * C], f32)  # [d0|d1] halves side by side
            nc.sync.dma_start(r(ndwT[:, 0:C]), r(ndwT_d[0:128, :]))
            nc.sync.dma_start(r(ndwT[:, C : 2 * C]), r(ndwT_d[128:256, :]))
            vecs = cpool.tile([C, 8], f32)
            nc.sync.dma_start(vecs[:], vec_d[:])
            ident = cpool.tile([128, 128], f32)
            nc.sync.dma_start(ident[:], ident_d[:])
            ones = cpool.tile([128, 128], f32)
            nc.sync.dma_start(r(ones[:]), r(ones_d[:]))
            ind4 = cpool.tile([C, GROUPS], f32)
            nc.sync.dma_start(ind4[:], ind4_d[:])
            indT = cpool.tile([GROUPS, C], f32)
            nc.sync.dma_start(indT[:], indT_d[:])

            gamma = vecs[:, 0:1]
            beta = vecs[:, 1:2]
            qb_s = vecs[:, 2:3]   # q_b * SC
            ndb = vecs[:, 3:4]    # nd_b
            pb = vecs[:, 4:5]     # proj_b

            for b in [bb % NB for bb in range(NB * reps)]:
                # ---- load x ----
                x_sb = xpool.tile([C, HW], f32, tag="x")
                for j in range(NCHUNK):
                    nc.sync.dma_start(
                        r(x_sb[:, 512 * j : 512 * (j + 1)]),
                        r(x_d[b, :, 512 * j : 512 * (j + 1)]),
                    )

                # ---- groupnorm stats ----
                bnbuf = spool.tile([C, 6 * NCHUNK], f32, tag="bnbuf")
                for j in range(NCHUNK):
                    nc.vector.bn_stats(
                        bnbuf[:, 6 * j : 6 * (j + 1)],
                        x_sb[:, 512 * j : 512 * (j + 1)],
                    )
                mv = spool.tile([C, 2], f32, tag="mv")  # per-channel mean, var
                nc.vector.bn_aggr(mv[:], bnbuf[:])
                ms = spool.tile([C, 2], f32, tag="ms")  # mean, E[x^2]
                nc.vector.tensor_copy(ms[:, 0:1], mv[:, 0:1])
                msq = spool.tile([C, 1], f32, tag="msq")
                nc.vector.tensor_mul(msq[:], mv[:, 0:1], mv[:, 0:1])
                nc.vector.tensor_add(ms[:, 1:2], msq[:], mv[:, 1:2])

                # group-average stats: [C,2] -> [G,2]
                g_ps = ppool.tile([GROUPS, 2], f32, tag="misc")
                nc.tensor.matmul(g_ps[:], lhsT=ind4[:], rhs=ms[:])
                gm = spool.tile([GROUPS, 2], f32, tag="gm")
                nc.vector.tensor_copy(gm[:], g_ps[:])
                gsq = spool.tile([GROUPS, 1], f32, tag="gsq")
                nc.vector.tensor_mul(gsq[:], gm[:, 0:1], gm[:, 0:1])
                gvar = spool.tile([GROUPS, 1], f32, tag="gvar")
                # var_g + eps = (E[x^2]_g + eps) - mean_g^2
                nc.vector.scalar_tensor_tensor(
                    out=gvar[:], in0=gm[:, 1:2], scalar=EPS, in1=gsq[:],
                    op0=Alu.add, op1=Alu.subtract,
                )
                gsd = spool.tile([GROUPS, 1], f32, tag="gsd")
                nc.scalar.activation(gsd[:], gvar[:], Act.Sqrt)
                gv = spool.tile([GROUPS, 2], f32, tag="gv")  # mean_g, rstd_g
                nc.vector.tensor_copy(gv[:, 0:1], gm[:, 0:1])
                nc.vector.reciprocal(gv[:, 1:2], gsd[:])

                # broadcast group stats back to channels: [G,2] -> [C,2]
                cb_ps = ppool.tile([C, 2], f32, tag="misc")
                nc.tensor.matmul(cb_ps[:], lhsT=indT[:], rhs=gv[:])
                cb = spool.tile([C, 2], f32, tag="cb")  # mean_c, rstd_c
                nc.vector.tensor_copy(cb[:], cb_ps[:])

                # folded affine: a2=gamma*sc*rstd ; b2=beta*sc - mean*a2
                a2 = spool.tile([C, 1], f32, tag="a2")
                nc.vector.scalar_tensor_tensor(
                    out=a2[:], in0=gamma, scalar=SC, in1=cb[:, 1:2],
                    op0=Alu.mult, op1=Alu.mult,
                )
                btmp = spool.tile([C, 1], f32, tag="btmp")
                nc.vector.tensor_mul(btmp[:], cb[:, 0:1], a2[:])
                b2 = spool.tile([C, 1], f32, tag="b2")
                nc.vector.scalar_tensor_tensor(
                    out=b2[:], in0=beta, scalar=SC, in1=btmp[:],
                    op0=Alu.mult, op1=Alu.subtract,
                )
                qw2 = spool.tile([C, C], f32, tag="qw2")
                nc.vector.tensor_scalar_mul(r(qw2[:]), qwT[:], a2[:])
                qb_ps = ppool.tile([C, 1], f32, tag="misc")
                nc.tensor.matmul(qb_ps[:], lhsT=qwT[:], rhs=b2[:])
                qb2 = spool.tile([C, 1], f32, tag="qb2")
                nc.vector.tensor_add(qb2[:], qb_ps[:], qb_s)

                # ---- q = qw2.T @ x + qb2 ----
                q_sb = xpool.tile([C, HW], f32, tag="q")
                for j in range(NCHUNK):
                    q_ps = ppool.tile([C, 512], f32, tag="mm")
                    nc.tensor.matmul(
                        q_ps[:], lhsT=r(qw2[:]),
                        rhs=r(x_sb[:, 512 * j : 512 * (j + 1)]),
                    )
                    nc.scalar.activation(
                        r(q_sb[:, 512 * j : 512 * (j + 1)]), q_ps[:],
                        Act.Identity, bias=qb2[:],
                    )

                # ---- kv = silu(nd) @ nd_w.T + nd_b, in [C, L] layout ----
                nd_sb = spool.tile([128, 4 * ND], f32, tag="ndl")
                for t in range(4):
                    nc.sync.dma_start(
                        nd_sb[:, ND * t : ND * (t + 1)],
                        nd_d[b, 128 * t : 128 * (t + 1), :],
                    )
                sig = spool.tile([128, 4 * ND], f32, tag="sig")
                for t in range(4):
                    nc.scalar.activation(
                        sig[:, ND * t : ND * (t + 1)],
                        nd_sb[:, ND * t : ND * (t + 1)], Act.Sigmoid,
                    )
                silu = spool.tile([128, 4 * ND], f32, tag="silu")
                nc.vector.tensor_mul(silu[:], sig[:], nd_sb[:])
                # transpose silu(nd) -> [ND, L] (two [128, 512] halves)
                ndT = spool.tile([128, 2 * L], f32, tag="ndT")
                for d in range(2):
                    ndT_ps = ppool.tile([128, L], f32, tag="mm")
                    for t in range(4):
                        nc.tensor.transpose(
                            ndT_ps[:, 128 * t : 128 * (t + 1)],
                            silu[:, ND * t + 128 * d : ND * t + 128 * (d + 1)],
                            ident[:],
                        )
                    nc.vector.tensor_copy(r(ndT[:, L * d : L * (d + 1)]), ndT_ps[:])
                kv_ps = ppool.tile([C, L], f32, tag="mm")
                nc.tensor.matmul(
                    kv_ps[:], lhsT=r(ndwT[:, 0:C]), rhs=r(ndT[:, 0:L]),
                    start=True, stop=False,
                )
                nc.tensor.matmul(
                    kv_ps[:], lhsT=r(ndwT[:, C : 2 * C]), rhs=r(ndT[:, L : 2 * L]),
                    start=False, stop=True,
                )
                kv_sb = spool.tile([C, L], f32, tag="kv")  # biased kv, [C, L]
                nc.vector.tensor_scalar_add(r(kv_sb[:]), kv_ps[:], ndb)

                # pkv[l, o] = sum_c kv[c, l] * proj_w.T[c, o]
                pkv = spool.tile([128, NL * 128], f32, tag="pkv")
                for li in range(NL):
                    pkv_ps = ppool.tile([128, 128], f32, tag="misc")
                    nc.tensor.matmul(
                        pkv_ps[:], lhsT=r(kv_sb[:, 128 * li : 128 * (li + 1)]),
                        rhs=r(pwT[:]),
                    )
                    nc.vector.tensor_copy(
                        r(pkv[:, 128 * li : 128 * (li + 1)]), pkv_ps[:]
                    )

                # ---- attention, 8 spatial chunks of 512 ----
                for j in range(NCHUNK):
                    qj = q_sb[:, 512 * j : 512 * (j + 1)]
                    exp_sb = apool.tile([128, NL * 512], f32, tag="exp")
                    for li in range(NL):
                        lg_ps = ppool.tile([128, 512], f32, tag="lg")
                        nc.tensor.matmul(
                            lg_ps[:],
                            lhsT=r(kv_sb[:, 128 * li : 128 * (li + 1)]),
                            rhs=r(qj),
                        )
                        nc.scalar.activation(
                            r(exp_sb[:, 512 * li : 512 * (li + 1)]), lg_ps[:],
                            Act.Exp,
                        )
                    sums_ps = ppool.tile([128, 512], f32, tag="sums", bufs=1)
                    for li in range(NL):
                        nc.tensor.matmul(
                            sums_ps[:], lhsT=r(ones[:]),
                            rhs=r(exp_sb[:, 512 * li : 512 * (li + 1)]),
                            start=(li == 0), stop=(li == NL - 1),
                        )
                    o2_ps = ppool.tile([128, 512], f32, tag="o2", bufs=1)
                    for li in range(NL):
                        nc.tensor.matmul(
                            o2_ps[:],
                            lhsT=r(pkv[:, 128 * li : 128 * (li + 1)]),
                            rhs=r(exp_sb[:, 512 * li : 512 * (li + 1)]),
                            start=(li == 0), stop=(li == NL - 1),
                        )
                    r_sb = apool.tile([128, 512], f32, tag="r")
                    nc.vector.reciprocal_approx_fast(out=r_sb[:], in_=sums_ps[:])
                    t_sb = apool.tile([128, 512], f32, tag="t")
                    nc.vector.tensor_mul(t_sb[:], o2_ps[:], r_sb[:])
                    o_sb = apool.tile([128, 512], f32, tag="o")
                    nc.vector.scalar_tensor_tensor(
                        out=o_sb[:], in0=t_sb[:], scalar=pb,
                        in1=x_sb[:, 512 * j : 512 * (j + 1)],
                        op0=Alu.add, op1=Alu.add,
                    )
                    nc.sync.dma_start(
                        y_d[b, :, 512 * j : 512 * (j + 1)], o_sb[:]
                    )

    nc.compile()
    return nc


def _get_nc():
    if "nc" not in _CACHE:
        _CACHE["nc"] = _build()
    return _CACHE["nc"]


def _prepare_in_maps(inputs):
    x = np.ascontiguousarray(inputs["x"], dtype=np.float32).reshape(B, C, HW)
    nd = np.ascontiguousarray(inputs["nd"], dtype=np.float32)
    q_w = np.asarray(inputs["q_w"], dtype=np.float32)
    q_b = np.asarray(inputs["q_b"], dtype=np.float32)
    nd_w = np.asarray(inputs["nd_w"], dtype=np.float32)
    nd_b = np.asarray(inputs["nd_b"], dtype=np.float32)
    proj_w = np.asarray(inputs["proj_w"], dtype=np.float32)
    proj_b = np.asarray(inputs["proj_b"], dtype=np.float32)
    gamma = np.asarray(inputs["gn_gamma"], dtype=np.float32)
    beta = np.asarray(inputs["gn_beta"], dtype=np.float32)

    vecs = np.zeros((C, 8), dtype=np.float32)
    vecs[:, 0] = gamma
    vecs[:, 1] = beta
    vecs[:, 2] = q_b * SC
    vecs[:, 3] = nd_b
    vecs[:, 4] = proj_b

    qwT = np.ascontiguousarray(q_w.T)
    pwT = np.ascontiguousarray(proj_w.T)
    ndwT = np.ascontiguousarray(nd_w.T)  # [ND, C]
    ident = np.eye(128, dtype=np.float32)
    ones = np.ones((128, 128), dtype=np.float32)
    cg = C // GROUPS
    ind4 = np.zeros((C, GROUPS), dtype=np.float32)
    ind4[np.arange(C), np.arange(C) // cg] = 1.0 / (cg)
    indT = np.zeros((GROUPS, C), dtype=np.float32)
    indT[np.arange(C) // cg, np.arange(C)] = 1.0

    shared = dict(
        qwT=qwT, pwT=pwT, ndwT=ndwT, vecs=vecs, ident=ident, ones=ones,
        ind4=ind4, indT=indT,
    )
    in_maps = []
    for i in range(NCORES):
        m = dict(shared)
        m["x"] = np.ascontiguousarray(x[NB * i : NB * (i + 1)])
        m["nd"] = np.ascontiguousarray(nd[NB * i : NB * (i + 1)])
        in_maps.append(m)
    return in_maps


def kernel(**inputs):
    from concourse.bass_utils import run_bass_kernel_spmd

    nc = _get_nc()
    in_maps = _prepare_in_maps(inputs)
    res = run_bass_kernel_spmd(nc, in_maps, core_ids=list(range(NCORES)))
    y = np.concatenate([res.results[i]["y"] for i in range(NCORES)], axis=0)
    return y.reshape(B, C, H, W)


# revision 9
# speedup vs baseline: 50.5481x; 50.5481x over previous
"""AttnBlock (GroupNorm -> 1x1 q conv -> cross-attn over silu(nd)@W -> 1x1 proj -> residual)
for Trainium2, 8 NeuronCores, pure data parallel (2 batches per core).

Math (per batch b):
  hn   = GroupNorm(x)                            [C, HW]
  q    = q_w @ hn + q_b                          [C, HW]   (C on partitions)
  kv   = silu(nd) @ nd_w.T + nd_b                [L, C]
  lgT  = kv^T q * C^-1/2                         [L, HW]   (logits, transposed)
  attn = softmax over L
  out  = proj_w @ (kv^T attn) + proj_b ; y = x + out

Device-side algebra (all biases/affines folded into matmuls):
  - GroupNorm affine folded into q_w:  q2_w[c,o] = q_w.T[c,o]*a[c],
    qb2[o] = sum_c q_w.T[c,o]*bshift[c] + q_b[o]*sc  where
    a[c] = gamma[c]*rstd[g(c)]*sc, bshift[c] = (beta[c]-mean[g]*gamma[c]*rstd[g])*sc.
    So q comes straight from raw x (one matmul + bias) and carries the C^-0.5 scale.
  - logits computed transposed: lgT[l,n] = sum_c kv[c,l]*q[c,n]  (kv biased).
  - softmax denom: ones[128,128] matmul over exp tiles -> sums replicated on
    all 128 partitions; reciprocal_approx_fast -> r[n].
  - attnV and proj fused: pkv[l,o] = sum_c kv[c,l]*proj_w.T[c,o];
    o2[o,n] = sum_l pkv[l,o]*exp[l,n].  Then
    y = (o2*r + proj_b) + x   (nd_b bias term materializes exactly through the
    r normalization: (kv+nd_b) makes o2 pick up (proj_w@nd_b)[o]*sums[n]).
  - float32r (TF32-class single-pass PE mode) for all N>=128 matmuls.
"""

import numpy as np

B, C, HW = 16, 128, 4096
H = W = 64
L, ND = 512, 256
GROUPS = 32
EPS = 1e-6
NCORES = 8
NB = B // NCORES  # batches per core
SC = float(C) ** -0.5
NCHUNK = HW // 512  # 8 spatial chunks of 512
NL = L // 128       # 4 l-chunks of 128

_CACHE = {}


def _build(reps=None):
    """Build the Bass module (one NeuronCore program, SPMD across 8 cores)."""
    from contextlib import ExitStack

    import concourse.bacc as bacc
    import concourse.bass as bass
    import concourse.mybir as mybir
    import concourse.tile as tile

    f32 = mybir.dt.float32
    f32r = mybir.dt.float32r
    Alu = mybir.AluOpType
    Act = mybir.ActivationFunctionType

    nc = bacc.Bacc(
        "TRN2",
        target_bir_lowering=False,
        debug=False,
        enable_asserts=False,
    )

    x_d = nc.dram_tensor("x", [NB, C, HW], f32, kind="ExternalInput").ap()
    nd_d = nc.dram_tensor("nd", [NB, L, ND], f32, kind="ExternalInput").ap()
    qwT_d = nc.dram_tensor("qwT", [C, C], f32, kind="ExternalInput").ap()
    pwT_d = nc.dram_tensor("pwT", [C, C], f32, kind="ExternalInput").ap()
    ndwT_d = nc.dram_tensor("ndwT", [ND, C], f32, kind="ExternalInput").ap()
    vec_d = nc.dram_tensor("vecs", [C, 8], f32, kind="ExternalInput").ap()
    ident_d = nc.dram_tensor("ident", [128, 128], f32, kind="ExternalInput").ap()
    ones_d = nc.dram_tensor("ones", [128, 128], f32, kind="ExternalInput").ap()
    ind4_d = nc.dram_tensor("ind4", [C, GROUPS], f32, kind="ExternalInput").ap()
    indT_d = nc.dram_tensor("indT", [GROUPS, C], f32, kind="ExternalInput").ap()
    y_d = nc.dram_tensor("y", [NB, C, HW], f32, kind="ExternalOutput").ap()

    import os
    use_f32r = os.environ.get("K_USE_F32R", "1") == "1"
    if reps is None:
        reps = int(os.environ.get("K_REPS", "1"))

    def r(ap):
        return ap.bitcast(f32r) if use_f32r else ap

    with tile.TileContext(nc) as tc:
        with ExitStack() as ctx:
            cpool = ctx.enter_context(tc.tile_pool(name="consts", bufs=1))
            xpool = ctx.enter_context(tc.tile_pool(name="xq", bufs=2))
            spool = ctx.enter_context(tc.tile_pool(name="small", bufs=2))
            apool = ctx.enter_context(tc.tile_pool(name="attn", bufs=2))
            ppool = ctx.enter_context(
                tc.tile_pool(name="psum", bufs=2, space="PSUM")
            )

            # ---- constants (loaded once) ----
            qwT = cpool.tile([C, C], f32)
            nc.sync.dma_start(qwT[:], qwT_d[:])
            pwT = cpool.tile([C, C], f32)
            nc.sync.dma_start(r(pwT[:]), r(pwT_d[:]))
            ndwT = cpool.tile([128, 2 * C], f32)  # [d0|d1] halves side by side
            nc.sync.dma_start(r(ndwT[:, 0:C]), r(ndwT_d[0:128, :]))
            nc.sync.dma_start(r(ndwT[:, C : 2 * C]), r(ndwT_d[128:256, :]))
            vecs = cpool.tile([C, 8], f32)
            nc.sync.dma_start(vecs[:], vec_d[:])
            ident = cpool.tile([128, 128], f32)
            nc.sync.dma_start(ident[:], ident_d[:])
            ones = cpool.tile([128, 128], f32)
            nc.sync.dma_start(r(ones[:]), r(ones_d[:]))
            ind4 = cpool.tile([C, GROUPS], f32)
            nc.sync.dma_start(ind4[:], ind4_d[:])
            indT = cpool.tile([GROUPS, C], f32)
            nc.sync.dma_start(indT[:], indT_d[:])

            gamma = vecs[:, 0:1]
            beta = vecs[:, 1:2]
            qb_s = vecs[:, 2:3]   # q_b * SC
            ndb = vecs[:, 3:4]    # nd_b
            pb = vecs[:, 4:5]     # proj_b

            for b in [bb % NB for bb in range(NB * reps)]:
                # ---- load x ----
                x_sb = xpool.tile([C, HW], f32, tag="x")
                for j in range(NCHUNK):
                    nc.sync.dma_start(
                        r(x_sb[:, 512 * j : 512 * (j + 1)]),
                        r(x_d[b, :, 512 * j : 512 * (j + 1)]),
                    )

                # ---- groupnorm stats ----
                bnbuf = spool.tile([C, 6 * NCHUNK], f32, tag="bnbuf")
                for j in range(NCHUNK):
                    nc.vector.bn_stats(
                        bnbuf[:, 6 * j : 6 * (j + 1)],
                        x_sb[:, 512 * j : 512 * (j + 1)],
                    )
                mv = spool.tile([C, 2], f32, tag="mv")  # per-channel mean, var
                nc.vector.bn_aggr(mv[:], bnbuf[:])
                ms = spool.tile([C, 2], f32, tag="ms")  # mean, E[x^2]
                nc.vector.tensor_copy(ms[:, 0:1], mv[:, 0:1])
                msq = spool.tile([C, 1], f32, tag="msq")
                nc.vector.tensor_mul(msq[:], mv[:, 0:1], mv[:, 0:1])
                nc.vector.tensor_add(ms[:, 1:2], msq[:], mv[:, 1:2])

                # group-average stats: [C,2] -> [G,2]
                g_ps = ppool.tile([GROUPS, 2], f32, tag="misc")
                nc.tensor.matmul(g_ps[:], lhsT=ind4[:], rhs=ms[:])
                gm = spool.tile([GROUPS, 2], f32, tag="gm")
                nc.vector.tensor_copy(gm[:], g_ps[:])
                gsq = spool.tile([GROUPS, 1], f32, tag="gsq")
                nc.vector.tensor_mul(gsq[:], gm[:, 0:1], gm[:, 0:1])
                gvar = spool.tile([GROUPS, 1], f32, tag="gvar")
                # var_g + eps = (E[x^2]_g + eps) - mean_g^2
                nc.vector.scalar_tensor_tensor(
                    out=gvar[:], in0=gm[:, 1:2], scalar=EPS, in1=gsq[:],
                    op0=Alu.add, op1=Alu.subtract,
                )
                gsd = spool.tile([GROUPS, 1], f32, tag="gsd")
                nc.scalar.activation(gsd[:], gvar[:], Act.Sqrt)
                gv = spool.tile([GROUPS, 2], f32, tag="gv")  # mean_g, rstd_g
                nc.vector.tensor_copy(gv[:, 0:1], gm[:, 0:1])
                nc.vector.reciprocal(gv[:, 1:2], gsd[:])

                # broadcast group stats back to channels: [G,2] -> [C,2]
                cb_ps = ppool.tile([C, 2], f32, tag="misc")
                nc.tensor.matmul(cb_ps[:], lhsT=indT[:], rhs=gv[:])
                cb = spool.tile([C, 2], f32, tag="cb")  # mean_c, rstd_c
                nc.vector.tensor_copy(cb[:], cb_ps[:])

                # folded affine: a2=gamma*sc*rstd ; b2=beta*sc - mean*a2
                a2 = spool.tile([C, 1], f32, tag="a2")
                nc.vector.scalar_tensor_tensor(
                    out=a2[:], in0=gamma, scalar=SC, in1=cb[:, 1:2],
                    op0=Alu.mult, op1=Alu.mult,
                )
                btmp = spool.tile([C, 1], f32, tag="btmp")
                nc.vector.tensor_mul(btmp[:], cb[:, 0:1], a2[:])
                b2 = spool.tile([C, 1], f32, tag="b2")
                nc.vector.scalar_tensor_tensor(
                    out=b2[:], in0=beta, scalar=SC, in1=btmp[:],
                    op0=Alu.mult, op1=Alu.subtract,
                )
                qw2 = spool.tile([C, C], f32, tag="qw2")
                nc.vector.tensor_scalar_mul(r(qw2[:]), qwT[:], a2[:])
                qb_ps = ppool.tile([C, 1], f32, tag="misc")
                nc.tensor.matmul(qb_ps[:], lhsT=qwT[:], rhs=b2[:])
                qb2 = spool.tile([C, 1], f32, tag="qb2")
                nc.vector.tensor_add(qb2[:], qb_ps[:], qb_s)

                # ---- q = qw2.T @ x + qb2 ----
                q_sb = xpool.tile([C, HW], f32, tag="q")
                for j in range(NCHUNK):
                    q_ps = ppool.tile([C, 512], f32, tag="mm")
                    nc.tensor.matmul(
                        q_ps[:], lhsT=r(qw2[:]),
                        rhs=r(x_sb[:, 512 * j : 512 * (j + 1)]),
                    )
                    nc.scalar.activation(
                        r(q_sb[:, 512 * j : 512 * (j + 1)]), q_ps[:],
                        Act.Identity, bias=qb2[:],
                    )

                # ---- kv = silu(nd) @ nd_w.T + nd_b, in [C, L] layout ----
                nd_sb = spool.tile([128, 4 * ND], f32, tag="ndl")
                for t in range(4):
                    nc.sync.dma_start(
                        nd_sb[:, ND * t : ND * (t + 1)],
                        nd_d[b, 128 * t : 128 * (t + 1), :],
                    )
                sig = spool.tile([128, 4 * ND], f32, tag="sig")
                for t in range(4):
                    nc.scalar.activation(
                        sig[:, ND * t : ND * (t + 1)],
                        nd_sb[:, ND * t : ND * (t + 1)], Act.Sigmoid,
                    )
                silu = spool.tile([128, 4 * ND], f32, tag="silu")
                nc.vector.tensor_mul(silu[:], sig[:], nd_sb[:])
                # transpose silu(nd) -> [ND, L] (two [128, 512] halves)
                ndT = spool.tile([128, 2 * L], f32, tag="ndT")
                for d in range(2):
                    ndT_ps = ppool.tile([128, L], f32, tag="mm")
                    for t in range(4):
                        nc.tensor.transpose(
                            ndT_ps[:, 128 * t : 128 * (t + 1)],
                            silu[:, ND * t + 128 * d : ND * t + 128 * (d + 1)],
                            ident[:],
                        )
                    nc.vector.tensor_copy(r(ndT[:, L * d : L * (d + 1)]), ndT_ps[:])
                kv_ps = ppool.tile([C, L], f32, tag="mm")
                nc.tensor.matmul(
                    kv_ps[:], lhsT=r(ndwT[:, 0:C]), rhs=r(ndT[:, 0:L]),
                    start=True, stop=False,
                )
                nc.tensor.matmul(
                    kv_ps[:], lhsT=r(ndwT[:, C : 2 * C]), rhs=r(ndT[:, L : 2 * L]),
                    start=False, stop=True,
                )
                kv_sb = spool.tile([C, L], f32, tag="kv")  # biased kv, [C, L]
                nc.vector.tensor_scalar_add(r(kv_sb[:]), kv_ps[:], ndb)

                # pkv[l, o] = sum_c kv[c, l] * proj_w.T[c, o]
                pkv = spool.tile([128, NL * 128], f32, tag="pkv")
                for li in range(NL):
                    pkv_ps = ppool.tile([128, 128], f32, tag="misc")
                    nc.tensor.matmul(
                        pkv_ps[:], lhsT=r(kv_sb[:, 128 * li : 128 * (li + 1)]),
                        rhs=r(pwT[:]),
                    )
                    nc.vector.tensor_copy(
                        r(pkv[:, 128 * li : 128 * (li + 1)]), pkv_ps[:]
                    )

                # ---- attention, 8 spatial chunks of 512 ----
                for j in range(NCHUNK):
                    qj = q_sb[:, 512 * j : 512 * (j + 1)]
                    exp_sb = apool.tile([128, NL * 512], f32, tag="exp")
                    for li in range(NL):
                        lg_ps = ppool.tile([128, 512], f32, tag="lg")
                        nc.tensor.matmul(
                            lg_ps[:],
                            lhsT=r(kv_sb[:, 128 * li : 128 * (li + 1)]),
                            rhs=r(qj),
                        )
                        nc.scalar.activation(
                            r(exp_sb[:, 512 * li : 512 * (li + 1)]), lg_ps[:],
                            Act.Exp,
                        )
                    sums_ps = ppool.tile([128, 512], f32, tag="sums", bufs=1)
                    for li in range(NL):
                        nc.tensor.matmul(
                            sums_ps[:], lhsT=r(ones[:]),
                            rhs=r(exp_sb[:, 512 * li : 512 * (li + 1)]),
                            start=(li == 0), stop=(li == NL - 1),
                        )
                    o2_ps = ppool.tile([128, 512], f32, tag="o2", bufs=1)
                    for li in range(NL):
                        nc.tensor.matmul(
                            o2_ps[:],
                            lhsT=r(pkv[:, 128 * li : 128 * (li + 1)]),
                            rhs=r(exp_sb[:, 512 * li : 512 * (li + 1)]),
                            start=(li == 0), stop=(li == NL - 1),
                        )
                    r_sb = apool.tile([128, 512], f32, tag="r")
                    nc.vector.reciprocal_approx_fast(out=r_sb[:], in_=sums_ps[:])
                    t_sb = apool.tile([128, 512], f32, tag="t")
                    nc.vector.tensor_mul(t_sb[:], o2_ps[:], r_sb[:])
                    o_sb = apool.tile([128, 512], f32, tag="o")
                    nc.vector.scalar_tensor_tensor(
                        out=o_sb[:], in0=t_sb[:], scalar=pb,
                        in1=x_sb[:, 512 * j : 512 * (j + 1)],
                        op0=Alu.add, op1=Alu.add,
                    )
                    nc.sync.dma_start(
                        y_d[b, :, 512 * j : 512 * (j + 1)], o_sb[:]
                    )

    nc.compile()
    return nc


def _get_nc(reps=None):
    key = ("nc", reps)
    if key not in _CACHE:
        _CACHE[key] = _build(reps)
    return _CACHE[key]


def _prepare_in_maps(inputs):
    x = np.ascontiguousarray(inputs["x"], dtype=np.float32).reshape(B, C, HW)
    nd = np.ascontiguousarray(inputs["nd"], dtype=np.float32)
    q_w = np.asarray(inputs["q_w"], dtype=np.float32)
    q_b = np.asarray(inputs["q_b"], dtype=np.float32)
    nd_w = np.asarray(inputs["nd_w"], dtype=np.float32)
    nd_b = np.asarray(inputs["nd_b"], dtype=np.float32)
    proj_w = np.asarray(inputs["proj_w"], dtype=np.float32)
    proj_b = np.asarray(inputs["proj_b"], dtype=np.float32)
    gamma = np.asarray(inputs["gn_gamma"], dtype=np.float32)
    beta = np.asarray(inputs["gn_beta"], dtype=np.float32)

    vecs = np.zeros((C, 8), dtype=np.float32)
    vecs[:, 0] = gamma
    vecs[:, 1] = beta
    vecs[:, 2] = q_b * SC
    vecs[:, 3] = nd_b
    vecs[:, 4] = proj_b

    qwT = np.ascontiguousarray(q_w.T)
    pwT = np.ascontiguousarray(proj_w.T)
    ndwT = np.ascontiguousarray(nd_w.T)  # [ND, C]
    ident = np.eye(128, dtype=np.float32)
    ones = np.ones((128, 128), dtype=np.float32)
    cg = C // GROUPS
    ind4 = np.zeros((C, GROUPS), dtype=np.float32)
    ind4[np.arange(C), np.arange(C) // cg] = 1.0 / (cg)
    indT = np.zeros((GROUPS, C), dtype=np.float32)
    indT[np.arange(C) // cg, np.arange(C)] = 1.0

    shared = dict(
        qwT=qwT, pwT=pwT, ndwT=ndwT, vecs=vecs, ident=ident, ones=ones,
        ind4=ind4, indT=indT,
    )
    in_maps = []
    for i in range(NCORES):
        m = dict(shared)
        m["x"] = np.ascontiguousarray(x[NB * i : NB * (i + 1)])
        m["nd"] = np.ascontiguousarray(nd[NB * i : NB * (i + 1)])
        in_maps.append(m)
    return in_maps


def kernel(**inputs):
    from concourse.bass_utils import run_bass_kernel_spmd

    nc = _get_nc()
    in_maps = _prepare_in_maps(inputs)
    res = run_bass_kernel_spmd(nc, in_maps, core_ids=list(range(NCORES)))
    y = np.concatenate([res.results[i]["y"] for i in range(NCORES)], axis=0)
    return y.reshape(B, C, H, W)
